# revision 13
# baseline (speedup 1.0000x reference)
"""Trainium2 Bass kernel for nn_PolyEpisodicRNNModel.

Model (per reference):
    h0 = 0.001;  scan over S steps:
        z = clip(tanh(h) @ w_hh0.T + x_t @ w_ih + b1, 0, 1)
        h = z**poly_power @ w_hh1.T + b2          (hs collects h)
    readout per step: out = relu(hs @ hl1_w.T + hl1_b) @ ro_w.T + ro_b

Strategy:
  - Data-parallel over batch: B=128 -> 16 per core on 8 NeuronCores.
  - Phase 0 precomputes xw = x @ w_ih + b1 for all steps as a batched
    GEMM (M=128) and stores it TRANSPOSED, so the scan's z-path adds it
    in cheap [128, 64] full-lane ops.
  - Scan: per step two H x H GEMMs with the weights as the float32r
    moving operand (N=512, 1 cycle/row) and small [128,16] activation
    tiles stationary.  State is carried transposed via PE transposes.
    Work is split into 512-column halves ordered so each half's
    post-processing overlaps the other half's matmuls (keeps PE gapless
    and the HAM clock warm).
  - hs is staged to DRAM transposed (float32r); a static phase-2
    readout (W-stationary, n=S*16 moving) computes the output.
  - All weight-layout transforms happen on the host (free w.r.t. HW).
"""

import os
import sys

for _p in ("/opt/trn_rl_repo",):
    if _p not in sys.path:
        sys.path.insert(0, _p)

import numpy as np

import concourse.bass as bass
import concourse.bacc as bacc
import concourse.mybir as mybir
import concourse.tile as tile
from concourse.bass import ds
from concourse.bass_utils import run_bass_kernel_spmd
from concourse.masks import make_identity

S, B, D, H, O = 512, 128, 256, 1024, 256
NCORES = 8
BS = B // NCORES            # 16 batch rows per core
KT = H // 128               # 8 contraction tiles over H
DTILES = D // 128           # 2 contraction tiles over D
OT = O // 128               # 2 output tiles
U = 32                      # scan steps unrolled per For_i iteration
SG = 8                      # hsT store group (steps per store batch)
HB = 4 * BS                 # 64: half of the transposed state's columns
N_TOT = S * BS              # 8192 readout columns per core
CHUNK = 512                 # readout moving-dim chunk
NCH = N_TOT // CHUNK        # 16 chunks

F32 = mybir.dt.float32
F32R = mybir.dt.float32r
AF = mybir.ActivationFunctionType
ALU = mybir.AluOpType
PE = mybir.EngineType.PE

TANH_H0 = float(np.tanh(0.001))

_BUILD_CACHE = {}
LAST_RESULTS = None


def _build(poly_power: int):
    nc = bacc.Bacc("TRN2", target_bir_lowering=False, debug=False,
                   num_devices=NCORES)

    # ---- DRAM parameters (host-prepped layouts) ----
    xT_d = nc.dram_tensor("xT", [DTILES, 128, BS, S], F32, kind="ExternalInput")
    w0T_d = nc.dram_tensor("w0T", [128, KT * H], F32, kind="ExternalInput")
    wih_d = nc.dram_tensor("wih", [128, DTILES * H], F32, kind="ExternalInput")
    w1T_d = nc.dram_tensor("w1T", [128, KT * H], F32, kind="ExternalInput")
    hl1T_d = nc.dram_tensor("hl1T", [128, KT * H], F32, kind="ExternalInput")
    roT_d = nc.dram_tensor("roT", [128, KT * O], F32, kind="ExternalInput")
    b1bc_d = nc.dram_tensor("b1bc", [128, H], F32, kind="ExternalInput")
    b2bc_d = nc.dram_tensor("b2bc", [BS, H], F32, kind="ExternalInput")
    hl1bc_d = nc.dram_tensor("hl1bc", [128, KT], F32, kind="ExternalInput")
    robc_d = nc.dram_tensor("robc", [128, OT], F32, kind="ExternalInput")

    outT_d = nc.dram_tensor("outT", [OT, 128, N_TOT], F32, kind="ExternalOutput")
    hsT_d = nc.dram_tensor("hsT", [KT, 128, N_TOT], F32R)   # scratch
    xwT_d = nc.dram_tensor("xwT", [128, S, KT, BS], F32)    # scratch

    with tile.TileContext(nc) as tc:
        with tc.tile_pool(name="const", bufs=1) as cpool:
            idn = cpool.tile([128, 128], F32, tag="idn")
            make_identity(nc, idn[:])
            idn16 = idn[0:BS, 0:BS]

            # loop-carried state: tanh(h).T in two halves [128, 4*BS]
            tanhT_state = [cpool.tile([128, HB], F32R, tag=f"tanhT{i}",
                                      name=f"tanhT{i}") for i in range(2)]

            with (
                tc.tile_pool(name="sw", bufs=1) as swpool,
            ):
                w0T = swpool.tile([128, KT * H], F32R, tag="w0T")
                w1T = swpool.tile([128, KT * H], F32R, tag="w1T")
                b2bc = swpool.tile([BS, H], F32, tag="b2bc")
                nc.sync.dma_start(out=b2bc[:], in_=b2bc_d[:])

                # ---- phase 0: xwT = (x @ w_ih + b1).T per step ----
                with (
                    tc.tile_pool(name="p0", bufs=2) as p0pool,
                    tc.tile_pool(name="p0s", bufs=1) as p0s,
                    tc.tile_pool(name="ps0", bufs=2, space="PSUM") as pp0,
                ):
                    wih = p0s.tile([128, DTILES * H], F32R, tag="wih")
                    b1bc = p0s.tile([128, H], F32, tag="b1bc")
                    nc.sync.dma_start(out=b1bc[:], in_=b1bc_d[:])
                    stgw = p0s.tile([128, DTILES * H], F32, tag="stgw")
                    nc.sync.dma_start(out=stgw[:], in_=wih_d[:])
                    nc.vector.tensor_copy(wih[:], stgw[:])

                    for b_ in range(BS):
                        for tc_ in range(S // 128):
                            xls = p0pool.tile([128, DTILES, 128], F32,
                                              tag="xls")
                            for dt_ in range(DTILES):
                                nc.sync.dma_start(
                                    out=xls[:, dt_, :],
                                    in_=xT_d[dt_, :, b_,
                                             tc_ * 128:(tc_ + 1) * 128])
                            xlr = p0pool.tile([128, DTILES, 128], F32R,
                                              tag="xlr")
                            nc.vector.tensor_copy(xlr[:], xls[:])
                            pxw = pp0.tile([128, H], F32, tag="pxw")
                            for dt_ in range(DTILES):
                                for hf in range(2):
                                    o0 = hf * 512
                                    nc.tensor.matmul(
                                        pxw[:, o0:o0 + 512],
                                        xlr[:, dt_, :],
                                        wih[:, dt_ * H + o0:
                                            dt_ * H + o0 + 512],
                                        start=(dt_ == 0),
                                        stop=(dt_ == DTILES - 1))
                            xw = p0pool.tile([128, H], F32, tag="xw")
                            nc.vector.tensor_add(xw[:], pxw[:], b1bc[:])
                            pxT = pp0.tile([128, KT, 128], F32, tag="pxT")
                            for kt in range(KT):
                                nc.tensor.transpose(
                                    pxT[:, kt, :],
                                    xw[:, kt * 128:(kt + 1) * 128],
                                    idn[:, :])
                            xwTs = p0pool.tile([128, KT, 128], F32,
                                               tag="xwTs")
                            nc.scalar.activation(xwTs[:], pxT[:], AF.Copy)
                            for kt in range(KT):
                                nc.sync.dma_start(
                                    out=xwT_d[:, tc_ * 128:(tc_ + 1) * 128,
                                              kt, b_],
                                    in_=xwTs[:, kt, :])

                # ---- load scan weights; init state ----
                with tc.tile_pool(name="stg", bufs=2) as stgpool:
                    for t_, d_ in ((w0T, w0T_d), (w1T, w1T_d)):
                        stg = stgpool.tile(list(t_.shape), F32, tag="stg")
                        nc.sync.dma_start(out=stg[:], in_=d_[:])
                        nc.vector.tensor_copy(t_[:], stg[:])
                    stg0 = stgpool.tile([128, HB], F32, tag="stg0")
                    nc.vector.memset(stg0[:], TANH_H0)
                    nc.vector.tensor_copy(tanhT_state[0][:], stg0[:])
                    nc.vector.tensor_copy(tanhT_state[1][:], stg0[:])

                # ---- phase 1: the scan ----
                with (
                    tc.tile_pool(name="work", bufs=1) as wpool,
                    tc.tile_pool(name="xin", bufs=4) as xpool,
                    tc.tile_pool(name="hout", bufs=2) as hpool,
                    tc.tile_pool(name="ps1", bufs=1, space="PSUM") as pp1,
                    tc.tile_pool(name="ps2", bufs=1, space="PSUM") as pp2,
                ):
                    with tc.For_i(0, S, U, hint_engines=(PE,)) as t0:
                        cur = tanhT_state
                        hTg = None
                        for u in range(U):
                            if u % SG == 0:
                                hTg = hpool.tile([128, SG, KT * BS], F32R,
                                                 tag="hTg", name="hTg")
                            # per-step xw (transposed, +b1 folded)
                            xwu = xpool.tile([128, KT * BS], F32, tag="xwu")
                            nc.sync.dma_start(
                                out=xwu[:],
                                in_=xwT_d[:, ds(t0 + u, 1), :, :])
                            # -- GEMM1: z = tanh(h) @ w0.T (+ xw later) --
                            pz = [pp1.tile([BS, 512], F32, tag=f"pz{i}",
                                           name=f"pz{i}") for i in range(2)]
                            for ho in range(2):   # early: needs state half 0
                                o0 = ho * 512
                                for kt in range(4):
                                    nc.tensor.matmul(
                                        pz[ho][:],
                                        cur[0][:, kt * BS:(kt + 1) * BS],
                                        w0T[:, kt * H + o0:
                                            kt * H + o0 + 512],
                                        start=(kt == 0), stop=False)
                            for ho in range(2):   # late: needs state half 1
                                o0 = ho * 512
                                for kt in range(4, KT):
                                    nc.tensor.matmul(
                                        pz[ho][:],
                                        cur[1][:, (kt - 4) * BS:
                                               (kt - 3) * BS],
                                        w0T[:, kt * H + o0:
                                            kt * H + o0 + 512],
                                        start=False, stop=(kt == KT - 1))
                            # z-post per half: raw copy, transpose, then
                            # full-lane add(xw)+clip+pow in [128, 64]
                            z2T = []
                            for hf in range(2):
                                zr = wpool.tile([BS, 512], F32,
                                                tag=f"zr{hf}")
                                nc.vector.tensor_copy(zr[:], pz[hf][:])
                                pzT = pp2.tile([128, HB], F32,
                                               tag=f"pzT{hf}")
                                for q in range(4):
                                    nc.tensor.transpose(
                                        pzT[:, q * BS:(q + 1) * BS],
                                        zr[0:BS, q * 128:(q + 1) * 128],
                                        idn16)
                                zc = wpool.tile([128, HB], F32,
                                                tag=f"zc{hf}")
                                nc.vector.tensor_add(
                                    zc[:], pzT[:],
                                    xwu[:, hf * HB:(hf + 1) * HB])
                                nc.vector.tensor_scalar(
                                    zc[:], zc[:], 0.0, 1.0,
                                    op0=ALU.max, op1=ALU.min)
                                zz = wpool.tile([128, HB], F32R,
                                                tag=f"z2T{hf}")
                                if poly_power == 2:
                                    nc.scalar.activation(zz[:], zc[:],
                                                         AF.Square)
                                elif poly_power == 1:
                                    nc.scalar.activation(zz[:], zc[:],
                                                         AF.Copy)
                                elif poly_power == 0:
                                    zp = wpool.tile([128, HB], F32,
                                                    tag=f"zp{hf}")
                                    nc.vector.memset(zp[:], 1.0)
                                    nc.vector.tensor_copy(zz[:], zp[:])
                                else:
                                    zp = wpool.tile([128, HB], F32,
                                                    tag=f"zp{hf}")
                                    nc.scalar.activation(zp[:], zc[:],
                                                         AF.Square)
                                    for _ in range(poly_power - 2):
                                        nc.vector.tensor_mul(zp[:], zp[:],
                                                             zc[:])
                                    nc.vector.tensor_copy(zz[:], zp[:])
                                z2T.append(zz)
                            # -- GEMM2: h = z^p @ w1.T --
                            ph = [pp1.tile([BS, 512], F32, tag=f"ph{i}",
                                           name=f"ph{i}") for i in range(2)]
                            for kh in range(2):
                                for ho in range(2):
                                    o0 = ho * 512
                                    for q in range(4):
                                        kt = kh * 4 + q
                                        nc.tensor.matmul(
                                            ph[ho][:],
                                            z2T[kh][:, q * BS:(q + 1) * BS],
                                            w1T[:, kt * H + o0:
                                                kt * H + o0 + 512],
                                            start=(kh == 0 and q == 0),
                                            stop=(kh == 1 and q == 3))
                            # h-post per half: +b2, transpose, stage, tanh
                            nxt = []
                            for hf in range(2):
                                o0 = hf * 512
                                hnt = wpool.tile([BS, 512], F32,
                                                 tag=f"hnt{hf}")
                                nc.vector.tensor_add(hnt[:], ph[hf][:],
                                                     b2bc[:, o0:o0 + 512])
                                phT = pp2.tile([128, HB], F32,
                                               tag=f"phT{hf}")
                                for q in range(4):
                                    nc.tensor.transpose(
                                        phT[:, q * BS:(q + 1) * BS],
                                        hnt[0:BS, q * 128:(q + 1) * 128],
                                        idn16)
                                nc.scalar.activation(
                                    hTg[:, u % SG, hf * HB:(hf + 1) * HB],
                                    phT[:], AF.Copy)
                                nx = tanhT_state[hf] if u == U - 1 else \
                                    wpool.tile([128, HB], F32R,
                                               tag=f"th{hf}", name=f"th{hf}")
                                nc.scalar.activation(nx[:], phT[:], AF.Tanh)
                                nxt.append(nx)
                            cur = nxt
                            if u % SG == SG - 1:
                                g0 = (u // SG) * SG
                                for kt in range(KT):
                                    nc.sync.dma_start(
                                        out=hsT_d[kt, :,
                                                  ds((t0 + g0) * BS,
                                                     SG * BS)],
                                        in_=hTg[:, :, kt * BS:(kt + 1) * BS])

            # ---- phase 2: readout ----
            with (
                tc.tile_pool(name="rw", bufs=1) as rwpool,
                tc.tile_pool(name="p2", bufs=2) as p2pool,
                tc.tile_pool(name="p2s", bufs=2) as p2spool,
                tc.tile_pool(name="ps3", bufs=2, space="PSUM") as pp3,
            ):
                hl1T = rwpool.tile([128, KT * H], F32R, tag="hl1T")
                roT = rwpool.tile([128, KT * O], F32R, tag="roT")
                hl1bc = rwpool.tile([128, KT], F32, tag="hl1bc")
                robc = rwpool.tile([128, OT], F32, tag="robc")
                for t_, d_ in ((hl1bc, hl1bc_d), (robc, robc_d)):
                    nc.sync.dma_start(out=t_[:], in_=d_[:])
                with tc.tile_pool(name="stg2", bufs=2) as stg2pool:
                    for t_, d_ in ((hl1T, hl1T_d), (roT, roT_d)):
                        stg = stg2pool.tile(list(t_.shape), F32, tag="stg2")
                        nc.sync.dma_start(out=stg[:], in_=d_[:])
                        nc.vector.tensor_copy(t_[:], stg[:])

                for c in range(NCH):
                    n0 = c * CHUNK
                    hsc = p2pool.tile([128, KT, CHUNK], F32R, tag="hsc")
                    for kt in range(KT):
                        nc.sync.dma_start(out=hsc[:, kt, :],
                                          in_=hsT_d[kt, :, n0:n0 + CHUNK])
                    hidT = p2pool.tile([128, KT, CHUNK], F32R, tag="hidT")
                    for gt in range(KT):
                        phid = pp3.tile([128, CHUNK], F32, tag="phid")
                        for ht in range(KT):
                            nc.tensor.matmul(
                                phid[:],
                                hl1T[:, ht * H + gt * 128:
                                     ht * H + (gt + 1) * 128],
                                hsc[:, ht, :],
                                start=(ht == 0), stop=(ht == KT - 1))
                        nc.scalar.activation(hidT[:, gt, :], phid[:], AF.Relu,
                                             bias=hl1bc[:, gt:gt + 1])
                    for ot in range(OT):
                        po = pp3.tile([128, CHUNK], F32, tag="po")
                        for gt in range(KT):
                            nc.tensor.matmul(
                                po[:],
                                roT[:, gt * O + ot * 128:
                                    gt * O + (ot + 1) * 128],
                                hidT[:, gt, :],
                                start=(gt == 0), stop=(gt == KT - 1))
                        osb = p2spool.tile([128, CHUNK], F32, tag="osb")
                        nc.vector.tensor_scalar_add(osb[:], po[:],
                                                    robc[:, ot:ot + 1])
                        nc.sync.dma_start(out=outT_d[ot, :, n0:n0 + CHUNK],
                                          in_=osb[:])

    nc.compile()
    return nc


def _get_nc(poly_power: int):
    if poly_power not in _BUILD_CACHE:
        _BUILD_CACHE[poly_power] = _build(poly_power)
    return _BUILD_CACHE[poly_power]


def _tile_kmajor(w, kt, width):
    """[K, width] -> [128, kt*width] with contraction tiled on partitions."""
    return np.ascontiguousarray(
        w.reshape(kt, 128, width).transpose(1, 0, 2).reshape(128, kt * width))


def kernel(x, w_ih, w_hh0, w_hh1, b1, b2, hl1_w, hl1_b, ro_w, ro_b,
           poly_power):
    global LAST_RESULTS
    x = np.asarray(x, np.float32)
    w_ih = np.asarray(w_ih, np.float32)
    w_hh0 = np.asarray(w_hh0, np.float32)
    w_hh1 = np.asarray(w_hh1, np.float32)
    b1 = np.asarray(b1, np.float32)
    b2 = np.asarray(b2, np.float32)
    hl1_w = np.asarray(hl1_w, np.float32)
    hl1_b = np.asarray(hl1_b, np.float32)
    ro_w = np.asarray(ro_w, np.float32)
    ro_b = np.asarray(ro_b, np.float32)
    p = int(poly_power)

    nc = _get_nc(p)

    # host layout prep (shared across cores)
    w0T = _tile_kmajor(np.ascontiguousarray(w_hh0.T), KT, H)
    wih = _tile_kmajor(w_ih, DTILES, H)
    w1T = _tile_kmajor(np.ascontiguousarray(w_hh1.T), KT, H)
    hl1T = _tile_kmajor(np.ascontiguousarray(hl1_w.T), KT, H)
    roT = _tile_kmajor(np.ascontiguousarray(ro_w.T), KT, O)
    b1bc = np.ascontiguousarray(np.broadcast_to(b1, (128, H)))
    b2bc = np.ascontiguousarray(np.broadcast_to(b2, (BS, H)))
    hl1bc = np.ascontiguousarray(hl1_b.reshape(KT, 128).T)
    robc = np.ascontiguousarray(ro_b.reshape(OT, 128).T)

    # x: [S, B, D] -> [DTILES, 128, B, S]  (batch-major, t inner)
    xT = np.ascontiguousarray(
        x.transpose(2, 1, 0).reshape(DTILES, 128, B, S))

    shared = dict(w0T=w0T, wih=wih, w1T=w1T, hl1T=hl1T, roT=roT,
                  b1bc=b1bc, b2bc=b2bc, hl1bc=hl1bc, robc=robc)
    in_maps = []
    for i in range(NCORES):
        m = dict(shared)
        m["xT"] = np.ascontiguousarray(xT[:, :, i * BS:(i + 1) * BS, :])
        in_maps.append(m)

    _trace = os.environ.get("KERNEL_TRACE", "") == "1"
    _kw = {}
    if _trace:
        _kw = dict(trace=True,
                   tmpdir=os.environ.get("KERNEL_TRACE_DIR") or None)
    res = run_bass_kernel_spmd(nc, in_maps, list(range(NCORES)), **_kw)
    LAST_RESULTS = res

    out = np.empty((S, B, O), np.float32)
    for i in range(NCORES):
        oT = res.results[i]["outT"]  # [OT, 128, S*BS]
        out[:, i * BS:(i + 1) * BS, :] = (
            oT.reshape(OT, 128, S, BS).transpose(2, 3, 0, 1)
            .reshape(S, BS, O))
    return out
